# revision 6
# baseline (speedup 1.0000x reference)
"""Decoder layer on 8 trn2 cores — head-parallel attention + token-parallel FFN.

v3 design (vs v2's data-parallel/redundant-KV):
- Each core owns 2 heads (host-sliced Wq/Wk/Wv cols, Wo rows) and runs
  attention for ALL 4096 tokens (2 batches) of those heads. Causal trimming
  is uniform across cores (SPMD-clean): only key tiles <= diagonal are
  computed -> 62.5% of full score/exp/ctx work, and K/V is projected once
  per token instead of 4x redundantly.
- Q projected for chunk 0 upfront; Q/K/V of chunk c+1 projected just-in-time
  as PE filler between the exp->ctx groups of attention call c (v2's proven
  exp-latency-hiding idiom).
- O-proj partials (2 heads) are ReduceScattered over all 8 cores in 4
  chunked fp16 RS calls (one per q-chunk, both batches), each overlapped
  with the next attention call. Rank r receives y^T for its 128 tokens per
  call in [p=d%128, dtile, tok] layout -> after 4 calls each core holds the
  v2 `yT` layout [128, 8, 512] for its 512 owned tokens.
- Owner adds bo + x (residual), then the v2 tail runs nearly verbatim:
  LN1 folded into FFN1, FFN2 split dog0/dog1, LN2, out. reciprocal_approx
  _fast replaces the 3.3us DVE reciprocals.

Token ownership: core r<4 -> batch 0, r>=4 -> batch 1; within its batch,
call qc gives it tokens 512*qc + 128*(r%4) + [0,128). Host reassembles.
"""

import sys

sys.path.insert(0, "/opt/trn_rl_repo")

import numpy as np

D = 1024          # d_model
H = 16            # heads
HD = 64           # head dim
DFF = 4096
EPS = 1e-6
B, S = 2, 2048
NCORES = 8
P = 128
NDT = D // P              # 8 d_model tiles
NFT = DFF // P            # 32 d_ff tiles
NTT = 8                   # token tiles of 512 (b*4 + qc)
QCH = 512
QT_T = 4                  # 128-key tiles per 512 chunk

_CACHE = {}


def _build():
    import concourse.bacc as bacc
    import concourse.mybir as mybir
    import concourse.tile as tile

    dt = mybir.dt
    F16 = dt.float16
    F32 = dt.float32
    AF = mybir.ActivationFunctionType
    OP = mybir.AluOpType

    nc = bacc.Bacc("TRN2", target_bir_lowering=False, debug=False,
                   num_devices=NCORES)

    # ---- I/O ----
    xt = nc.dram_tensor("xt", [NTT, P, NDT, QCH], F16, kind="ExternalInput")
    wq_d = nc.dram_tensor("wq", [P, NDT, P], F16, kind="ExternalInput")
    wk_d = nc.dram_tensor("wk", [P, NDT, P], F16, kind="ExternalInput")
    wv_d = nc.dram_tensor("wv", [P, NDT, P], F16, kind="ExternalInput")
    wo_d = nc.dram_tensor("wo", [P, NDT, P], F16, kind="ExternalInput")
    w1 = nc.dram_tensor("w1", [NFT, P, NDT, P], F16, kind="ExternalInput")
    w2 = nc.dram_tensor("w2", [2, NFT, P, QCH], F16, kind="ExternalInput")
    ones_d = nc.dram_tensor("ones", [P, P], F16, kind="ExternalInput")
    tri_d = nc.dram_tensor("tri", [P, QT_T, QCH], F16, kind="ExternalInput")
    sel2_d = nc.dram_tensor("sel2", [HD, P], F32, kind="ExternalInput")
    bqk_d = nc.dram_tensor("bqk", [P, 2], F32, kind="ExternalInput")
    bvb_d = nc.dram_tensor("bvb", [P, P], F16, kind="ExternalInput")
    xres_d = nc.dram_tensor("xres", [P, NDT, QCH], F16, kind="ExternalInput")
    # cols8 order: bq, bk, bo, b2, g1, be1, g2, be2, -g1, -g2 (v2 format)
    cols8_d = nc.dram_tensor("cols8", [P, 10, NDT], F32,
                             kind="ExternalInput")
    cols32_d = nc.dram_tensor("cols32", [P, 2, NFT], F32,
                              kind="ExternalInput")
    out = nc.dram_tensor("out", [NDT, P, QCH], F16, kind="ExternalOutput")

    from contextlib import ExitStack

    with tile.TileContext(nc) as tc:
        with ExitStack() as _stk:
            def pool(name, bufs, space="SBUF"):
                return _stk.enter_context(
                    tc.tile_pool(name=name, bufs=bufs, space=space))

            consts = pool("consts", 1)
            xp = pool("xp", 5)          # x^T tiles (rotating)
            qkv = pool("qkv", 1)        # qT, kT, v
            expp = pool("expp", 3)
            ctxtp = pool("ctxtp", 2)
            stp = pool("stp", 2)        # RS staging (4KB half-tiles)
            wst = pool("wst", 4)        # streamed w1 tiles
            wmv = pool("wmv", 3)        # streamed w2 tiles
            yp = pool("yp", 1)          # yT then outT
            hp = pool("hp", 1)
            y2p = pool("y2p", 1)
            ffp = pool("ffp", 1)
            lns = pool("lns", 1)
            sm = pool("sm", 2)
            dram = pool("dram", 8, space="DRAM")
            ps_q = pool("ps_q", 2, space="PSUM")
            ps_sc = pool("ps_sc", 2, space="PSUM")
            ps_ctx = pool("ps_ctx", 1, space="PSUM")

            # ---- x tiles first (critical path), consts via gpsimd ----
            x_sb = {}
            for tt in (0, 4, 1, 5, 2, 6, 3, 7):
                x_sb[tt] = xp.tile([P, NDT, QCH], F16, tag="x",
                                   name=f"x{tt}")
                if tt == 0:
                    # sliced so Q(0)'s first matmuls start sooner
                    for k2 in range(4):
                        nc.sync.dma_start(
                            x_sb[tt][:, 2 * k2:2 * k2 + 2, :],
                            xt[tt][:, 2 * k2:2 * k2 + 2, :])
                else:
                    nc.sync.dma_start(x_sb[tt][:], xt[tt])
            wq_sb = consts.tile([P, NDT, P], F16, tag="wq")
            nc.gpsimd.dma_start(out=wq_sb[:], in_=wq_d[:])
            wk_sb = consts.tile([P, NDT, P], F16, tag="wk")
            nc.gpsimd.dma_start(out=wk_sb[:], in_=wk_d[:])
            wv_sb = consts.tile([P, NDT, P], F16, tag="wv")
            nc.gpsimd.dma_start(out=wv_sb[:], in_=wv_d[:])
            bqk = consts.tile([P, 2], F32, tag="bqk")
            nc.gpsimd.dma_start(out=bqk[:], in_=bqk_d[:])
            bvb = consts.tile([P, P], F16, tag="bvb")
            nc.gpsimd.dma_start(out=bvb[:], in_=bvb_d[:])
            ones16 = consts.tile([P, P], F16, tag="ones")
            nc.gpsimd.dma_start(out=ones16[:], in_=ones_d[:])
            tri = consts.tile([P, QT_T, QCH], F16, tag="tri")
            nc.gpsimd.dma_start(out=tri[:], in_=tri_d[:])
            sel2 = consts.tile([HD, P], F32, tag="sel2")
            nc.gpsimd.dma_start(out=sel2[:], in_=sel2_d[:])
            wo_sb = consts.tile([P, NDT, P], F16, tag="wo")
            nc.gpsimd.dma_start(out=wo_sb[:], in_=wo_d[:])
            cols8 = consts.tile([P, 10, NDT], F32, tag="cols8")
            nc.gpsimd.dma_start(out=cols8[:], in_=cols8_d[:])
            cols32 = consts.tile([P, 2, NFT], F32, tag="cols32")
            nc.gpsimd.dma_start(out=cols32[:], in_=cols32_d[:])
            xres = consts.tile([P, NDT, QCH], F16, tag="xres")
            nc.gpsimd.dma_start(out=xres[:], in_=xres_d[:])
            bo_c, b2_c = cols8[:, 2], cols8[:, 3]
            g1_c, be1_c, g2_c, be2_c = [cols8[:, 4 + j] for j in range(4)]
            g1n_c, g2n_c = cols8[:, 8], cols8[:, 9]
            c1_c = cols32[:, 0]
            s1n_c = cols32[:, 1]
            eps_sb = consts.tile([P, 1], F32, tag="eps")
            nc.vector.memset(eps_sb[:], EPS)

            # ---- projection targets ----
            qT = qkv.tile([P, NTT, QCH], F16, tag="qT", name="qT")
            kT = qkv.tile([P, NTT, QCH], F16, tag="kT", name="kT")
            # v: [tok%128, 32 key tiles, 2*(HD+1)] with ones at cols 64, 129
            v = qkv.tile([P, 4 * NTT, 2 * (HD + 1)], F16, tag="v", name="v")
            nc.vector.memset(v[:, :, HD:HD + 1], 1.0)
            nc.vector.memset(v[:, :, 2 * HD + 1:2 * HD + 2], 1.0)

            def emit_q(tt):
                pq = ps_q.tile([P, QCH], F32, tag="ps_q", name=f"pq{tt}")
                for ko in range(NDT):
                    nc.tensor.matmul(pq[:], wq_sb[:, ko, :],
                                     x_sb[tt][:, ko, :],
                                     start=(ko == 0), stop=(ko == NDT - 1))
                nc.vector.tensor_scalar(
                    out=qT[:, tt, :], in0=pq[:],
                    scalar1=bqk[:, 0:1], scalar2=None, op0=OP.add)

            def emit_k(tt):
                pk = ps_q.tile([P, QCH], F32, tag="ps_q", name=f"pk{tt}")
                for ko in range(NDT):
                    nc.tensor.matmul(pk[:], wk_sb[:, ko, :],
                                     x_sb[tt][:, ko, :],
                                     start=(ko == 0), stop=(ko == NDT - 1))
                nc.vector.tensor_scalar(
                    out=kT[:, tt, :], in0=pk[:],
                    scalar1=bqk[:, 1:2], scalar2=None, op0=OP.add)

            def emit_v(tt, ts):
                """V for tokens [512*tt + 128*ts ...): x-stationary matmuls."""
                pv = ps_q.tile([P, P], F32, tag="ps_q", name=f"pv{tt}_{ts}")
                for ko in range(NDT):
                    nc.tensor.matmul(
                        pv[:], x_sb[tt][:, ko, ts * P:(ts + 1) * P],
                        wv_sb[:, ko, :],
                        start=(ko == 0), stop=(ko == NDT - 1))
                kt = 4 * tt + ts
                for h in range(2):
                    nc.vector.tensor_tensor(
                        v[:, kt, (HD + 1) * h:(HD + 1) * h + HD],
                        pv[:, HD * h:HD * h + HD],
                        bvb[:, HD * h:HD * h + HD], OP.add)

            def qkv_units(qc):
                """JIT projection closures for chunk qc (both batches)."""
                units = []
                for b in range(2):
                    tt = 4 * b + qc
                    units.append(lambda tt=tt: emit_q(tt))
                    units.append(lambda tt=tt: emit_k(tt))
                    for ts in range(4):
                        units.append(lambda tt=tt, ts=ts: emit_v(tt, ts))
                return units

            # ---- upfront: Q/K/V for chunk 0 (both batches) ----
            for u in qkv_units(0):
                u()

            # ---- RS buffers ----
            rs_in = [dram.tile([NCORES, P, NDT, P], F16, name=f"rsi{qc}")
                     for qc in range(2)]
            rs_out = [dram.tile([P, NDT, P], F16,
                                name=f"rso{qc}") for qc in range(2)]
            rs_inh = {qc: [dram.tile([NCORES, P, 4, P], F16,
                                     name=f"rsi{qc}{h}") for h in range(2)]
                      for qc in (2, 3)}
            rs_outh = {qc: [dram.tile([P, 4, P], F16,
                                      name=f"rso{qc}{h}") for h in range(2)]
                       for qc in (2, 3)}

            yT = yp.tile([P, NDT, QCH], F16, tag="y", name="yT")

            # ---- attention calls ----
            pend_oproj = []
            pend_rb = []
            for qc in range(4):
                fillers = qkv_units(qc + 1) if qc < 3 else []
                fillers = pend_oproj + fillers
                pend_oproj = []
                nkg = 2 * (qc + 1)          # 2-ktile score groups per batch
                ctxT = ctxtp.tile([P, 2, QCH], F16, tag="ctxT",
                                  name=f"ctxT{qc}")
                for b in range(2):
                    qtt = 4 * b + qc
                    pcs = ps_ctx.tile([HD + 1, 2, QCH], F32, tag="ps_ctx",
                                      name=f"pcs{qc}_{b}")
                    for j in range(4 * (qc + 1)):
                        # one psc per key tile, BOTH heads side by side in
                        # bank-aligned halves: scores(j+1) can issue while
                        # exp(j) drains -> no scalar-engine idle per group
                        ktt = 4 * b + j // 4
                        ks = j % 4
                        psc = ps_sc.tile([P, 2, QCH], F32, tag="ps_sc",
                                         name=f"psc{qc}_{b}_{j}")
                        for h in range(2):
                            bp = HD * h
                            nc.tensor.matmul(
                                psc[:, h, :],
                                kT[bp:bp + HD, ktt, ks * P:(ks + 1) * P],
                                qT[bp:bp + HD, qtt, :],
                                start=True, stop=True,
                                tile_position=(bp, 0))
                        ex = expp.tile([P, 2, QCH], F16, tag="exp",
                                       name=f"ex{qc}_{b}_{j}")
                        nc.scalar.activation(out=ex[:], in_=psc[:],
                                             func=AF.Exp, scale=0.125)
                        if j >= 4 * qc:     # diagonal chunk: tri mask
                            jd = j - 4 * qc
                            for h in range(2):
                                nc.vector.tensor_tensor(
                                    ex[:, h, :], ex[:, h, :],
                                    tri[:, jd, :], OP.mult)
                        if fillers and j % 2 == 1:
                            fillers.pop(0)()
                        for h in range(2):
                            hb = (HD + 1) * h
                            kt = 16 * b + j
                            nc.tensor.matmul(
                                pcs[:, h, :],
                                v[:, kt, hb:hb + HD + 1],
                                ex[:, h, :],
                                start=(j == 0), stop=(j == 4 * qc + 3))
                    # normalize this batch's ctx
                    den = sm.tile([HD, QCH], F32, tag="den",
                                  name=f"den{qc}_{b}")
                    nc.vector.memset(den[:], 1.0)
                    for h in range(2):
                        nc.vector.tensor_copy(out=den[32 * h:32 * h + 1, :],
                                              in_=pcs[HD:HD + 1, h, :])
                    for h in range(2):
                        nc.vector.tensor_copy(
                            out=ctxT[HD * h:HD * h + HD, b, :],
                            in_=pcs[0:HD, h, :])
                    denr = sm.tile([HD, QCH], F32, tag="den",
                                   name=f"denr{qc}_{b}")
                    nc.vector.reciprocal_approx_fast(out=denr[:],
                                                     in_=den[:])
                    prc = ps_q.tile([P, QCH], F32, tag="ps_q",
                                    name=f"prc{qc}_{b}")
                    nc.tensor.matmul(prc[:], sel2[:], denr[:],
                                     start=True, stop=True)
                    prc_sb = sm.tile([P, QCH], F16, tag="sq",
                                     name=f"prcs{qc}_{b}")
                    nc.vector.tensor_copy(out=prc_sb[:], in_=prc[:])
                    nc.vector.tensor_tensor(
                        ctxT[:, b, :], ctxT[:, b, :], prc_sb[:], OP.mult)
                for f in fillers:
                    f()

                def emit_oproj(qc=qc, ctxT=ctxT):
                    # qc<3: one 2MB RS, staging half-split (4KB st tiles).
                    # qc==3: d-half-major order with TWO 1MB collectives so
                    # the first doorbell fires ~4us earlier and each call's
                    # latency is roughly halved -> less exposed tail.
                    split = (qc >= 2)
                    for hf in range(2):
                        for b in range(2):
                            st = stp.tile([P, 4, 4, P], F16, tag="st",
                                          name=f"st{qc}_{b}_{hf}")
                            for d4 in range(4):
                                do = 4 * hf + d4
                                po = ps_q.tile([P, QCH], F32, tag="ps_q",
                                               name=f"po{qc}_{do}_{b}")
                                nc.tensor.matmul(po[:], wo_sb[:, do, :],
                                                 ctxT[:, b, :],
                                                 start=True, stop=True)
                                nc.vector.tensor_copy(
                                    out=st[:, :, d4, :],
                                    in_=po[:].rearrange(
                                        "p (tb tl) -> p tb tl", tl=P))
                            dst = (rs_inh[qc][hf] if split else
                                   rs_in[qc][:, :, 4 * hf:4 * hf + 4, :])
                            nc.sync.dma_start(
                                dst[4 * b:4 * b + 4].rearrange(
                                    "tb p do tl -> p tb do tl") if split
                                else rs_in[qc][4 * b:4 * b + 4, :,
                                               4 * hf:4 * hf + 4, :]
                                .rearrange("tb p do tl -> p tb do tl"),
                                st[:])
                        if split:
                            nc.gpsimd.collective_compute(
                                "ReduceScatter", mybir.AluOpType.add,
                                replica_groups=[list(range(NCORES))],
                                ins=[rs_inh[qc][hf].opt()],
                                outs=[rs_outh[qc][hf].opt()],
                            )
                            hh = hf
                            pend_rb.append(lambda hh=hh, qc=qc:
                                           nc.gpsimd.dma_start(
                                out=yT[:, 4 * hh:4 * hh + 4,
                                       qc * P:(qc + 1) * P],
                                in_=rs_outh[qc][hh][:]))
                    if not split:
                        nc.gpsimd.collective_compute(
                            "ReduceScatter", mybir.AluOpType.add,
                            replica_groups=[list(range(NCORES))],
                            ins=[rs_in[qc].opt()], outs=[rs_out[qc].opt()],
                        )
                        pend_rb.append(lambda qc=qc: nc.gpsimd.dma_start(
                            out=yT[:, :, qc * P:(qc + 1) * P],
                            in_=rs_out[qc][:]))

                if qc == 0:
                    emit_oproj()          # keep RS_0 as early as possible
                else:
                    pend_oproj.append(emit_oproj)
                if qc == 3:
                    for f in pend_oproj:
                        f()
                    pend_oproj = []
                    # readbacks after ALL triggers: none of them can delay
                    # a later doorbell on the gpsimd FIFO
                    for rbf in pend_rb:
                        rbf()

            # ---- residual (bo folded into xres host-side) + LN1 stats ----
            ps1 = ps_sc.tile([P, 2, QCH], F32, tag="ps_sc", name="ps12")
            for do in range(NDT):
                eng = nc.vector if do % 2 == 0 else nc.gpsimd
                eng.tensor_tensor(yT[:, do, :], yT[:, do, :],
                                  xres[:, do, :], OP.add)
                sq = sm.tile([P, QCH], F16, tag="sq")
                eng.tensor_tensor(sq[:], yT[:, do, :], yT[:, do, :],
                                  OP.mult)
                nc.tensor.matmul(ps1[:, 0, :], ones16[:], yT[:, do, :],
                                 start=(do == 0), stop=(do == NDT - 1))
                nc.tensor.matmul(ps1[:, 1, :], ones16[:], sq[:],
                                 start=(do == 0), stop=(do == NDT - 1))

            # ---- LN1 stats chain (apply is folded into FFN1) ----
            mean = lns.tile([P, QCH], F32, tag="mean")
            nc.vector.tensor_scalar(out=mean[:], in0=ps1[:, 0, :],
                                    scalar1=1.0 / D, scalar2=None,
                                    op0=OP.mult)
            m2 = sm.tile([P, QCH], F16, tag="sq", name="m2")
            nc.vector.tensor_tensor(m2[:], mean[:], mean[:], OP.mult)
            var = lns.tile([P, QCH], F32, tag="var")
            nc.vector.scalar_tensor_tensor(
                out=var[:], in0=ps1[:, 1, :], scalar=1.0 / D, in1=m2[:],
                op0=OP.mult, op1=OP.subtract)
            sstd = lns.tile([P, QCH], F32, tag="sstd")
            nc.scalar.activation(out=sstd[:], in_=var[:], func=AF.Sqrt,
                                 bias=eps_sb[:], scale=1.0)
            rstd = lns.tile([P, QCH], F32, tag="rstd")
            nc.vector.reciprocal_approx_fast(out=rstd[:], in_=sstd[:])
            rstd16 = lns.tile([P, QCH], F16, tag="rstd16")
            nc.vector.tensor_copy(out=rstd16[:], in_=rstd[:])
            mr = lns.tile([P, QCH], F32, tag="mr")
            nc.vector.tensor_tensor(mr[:], mean[:], rstd[:], OP.mult)

            # h (LN1 output) for the 2nd residual
            hT = hp.tile([P, NDT, QCH], F16, tag="h")
            for do in range(NDT):
                b1t = sm.tile([P, QCH], F16, tag="hu", name=f"b1{do}")
                nc.vector.tensor_scalar(
                    out=b1t[:], in0=mr[:],
                    scalar1=g1n_c[:, do:do + 1],
                    scalar2=be1_c[:, do:do + 1],
                    op0=OP.mult, op1=OP.add)
                u = sm.tile([P, QCH], F16, tag="hu", name=f"u{do}")
                nc.gpsimd.tensor_tensor(u[:], yT[:, do, :], rstd16[:],
                                        OP.mult)
                nc.vector.scalar_tensor_tensor(
                    out=hT[:, do, :], in0=u[:],
                    scalar=g1_c[:, do:do + 1], in1=b1t[:],
                    op0=OP.mult, op1=OP.add)

            # ---- FFN1 with FFN2 dog=0 skewed one ft behind ----
            ff = ffp.tile([P, NFT, QCH], F16, tag="ff")
            pds0 = [ps_sc.tile([P, 2, QCH], F32, tag="ps_sc",
                               name=f"pd0{i}") for i in range(2)]

            def stream_w8(dram_ap):
                t = wst.tile([P, NDT, P], F16, tag="w8")
                nc.sync.dma_start(t[:], dram_ap)
                return t

            def stream_w512(dram_ap):
                t = wmv.tile([P, QCH], F16, tag="w512")
                nc.scalar.dma_start(out=t[:], in_=dram_ap)
                return t

            def emit_ffn1(ft):
                w1_t = stream_w8(w1[ft])
                pfp = ps_q if ft % 2 == 0 else ps_ctx
                pf = pfp.tile([P, QCH], F32, tag=pfp.name, name=f"pf{ft}")
                for k in range(NDT):
                    nc.tensor.matmul(pf[:], w1_t[:, k, :], yT[:, k, :],
                                     start=(k == 0), stop=(k == NDT - 1))
                A = sm.tile([P, QCH], F16, tag="sq", name=f"A{ft}")
                nc.vector.tensor_tensor(A[:], pf[:], rstd16[:], OP.mult)
                Bv = sm.tile([P, QCH], F16, tag="sq", name=f"B{ft}")
                nc.vector.scalar_tensor_tensor(
                    out=Bv[:], in0=mr[:], scalar=s1n_c[:, ft:ft + 1],
                    in1=A[:], op0=OP.mult, op1=OP.add)
                nc.scalar.activation(out=ff[:, ft, :], in_=Bv[:],
                                     func=AF.Relu,
                                     bias=c1_c[:, ft:ft + 1], scale=1.0)

            def emit_ffn2_dog0(ft):
                w2_t = stream_w512(w2[0, ft])
                for d4 in range(4):
                    nc.tensor.matmul(
                        pds0[d4 // 2][:, d4 % 2, :],
                        w2_t[:, d4 * P:(d4 + 1) * P], ff[:, ft, :],
                        start=(ft == 0), stop=(ft == NFT - 1))

            for ft in range(NFT):
                emit_ffn1(ft)
                if ft >= 1:
                    emit_ffn2_dog0(ft - 1)
            emit_ffn2_dog0(NFT - 1)

            # ---- y2 (dog=0 half) + LN2 stats started ----
            y2T = y2p.tile([P, NDT, QCH], F16, tag="y2")
            ps2 = ps_ctx.tile([P, 2, QCH], F32, tag="ps_ctx", name="ps2")
            for d4 in range(4):
                nc.vector.scalar_tensor_tensor(
                    out=y2T[:, d4, :], in0=pds0[d4 // 2][:, d4 % 2, :],
                    scalar=b2_c[:, d4:d4 + 1], in1=hT[:, d4, :],
                    op0=OP.add, op1=OP.add)
                sq2 = sm.tile([P, QCH], F16, tag="sq", name=f"s2{d4}")
                nc.vector.tensor_tensor(sq2[:], y2T[:, d4, :],
                                        y2T[:, d4, :], OP.mult)
                nc.tensor.matmul(ps2[:, 0, :], ones16[:], y2T[:, d4, :],
                                 start=(d4 == 0), stop=False)
                nc.tensor.matmul(ps2[:, 1, :], ones16[:], sq2[:],
                                 start=(d4 == 0), stop=False)

            # ---- FFN2 dog=1 half ----
            pds1 = [ps_sc.tile([P, 2, QCH], F32, tag="ps_sc",
                               name=f"pd1{i}") for i in range(2)]
            for k in range(NFT):
                w2_t = stream_w512(w2[1, k])
                for d4 in range(4):
                    nc.tensor.matmul(
                        pds1[d4 // 2][:, d4 % 2, :],
                        w2_t[:, d4 * P:(d4 + 1) * P], ff[:, k, :],
                        start=(k == 0), stop=(k == NFT - 1))
            for d4 in range(4):
                do = 4 + d4
                nc.vector.scalar_tensor_tensor(
                    out=y2T[:, do, :], in0=pds1[d4 // 2][:, d4 % 2, :],
                    scalar=b2_c[:, do:do + 1], in1=hT[:, do, :],
                    op0=OP.add, op1=OP.add)
                sq2 = sm.tile([P, QCH], F16, tag="sq", name=f"s2{do}")
                nc.vector.tensor_tensor(sq2[:], y2T[:, do, :],
                                        y2T[:, do, :], OP.mult)
                nc.tensor.matmul(ps2[:, 0, :], ones16[:], y2T[:, do, :],
                                 start=False, stop=(d4 == 3))
                nc.tensor.matmul(ps2[:, 1, :], ones16[:], sq2[:],
                                 start=False, stop=(d4 == 3))

            # ---- LN2 chain + apply (split Vector / GpSimd) + DMA out ----
            mean2 = lns.tile([P, QCH], F32, tag="mean")
            nc.vector.tensor_scalar(out=mean2[:], in0=ps2[:, 0, :],
                                    scalar1=1.0 / D, scalar2=None,
                                    op0=OP.mult)
            m22 = sm.tile([P, QCH], F16, tag="sq", name="m22")
            nc.vector.tensor_tensor(m22[:], mean2[:], mean2[:], OP.mult)
            var2 = lns.tile([P, QCH], F32, tag="var")
            nc.vector.scalar_tensor_tensor(
                out=var2[:], in0=ps2[:, 1, :], scalar=1.0 / D, in1=m22[:],
                op0=OP.mult, op1=OP.subtract)
            sstd2 = lns.tile([P, QCH], F32, tag="sstd", name="sstd2")
            nc.scalar.activation(out=sstd2[:], in_=var2[:], func=AF.Sqrt,
                                 bias=eps_sb[:], scale=1.0)
            rstd2 = lns.tile([P, QCH], F32, tag="rstd", name="rstd2")
            nc.vector.reciprocal_approx_fast(out=rstd2[:], in_=sstd2[:])
            rstd216 = lns.tile([P, QCH], F16, tag="rstd16", name="rstd216")
            nc.vector.tensor_copy(out=rstd216[:], in_=rstd2[:])
            mr216 = lns.tile([P, QCH], F16, tag="mr16")
            nc.vector.tensor_tensor(mr216[:], mean2[:], rstd2[:], OP.mult)
            outT = yp.tile([P, NDT, QCH], F16, tag="y", name="outT")
            for do in range(NDT):
                eng = nc.vector if do % 2 == 0 else nc.gpsimd
                u = sm.tile([P, QCH], F16, tag="hu", name=f"o{do}")
                eng.tensor_tensor(u[:], y2T[:, do, :], rstd216[:],
                                  OP.mult)
                eng.tensor_tensor(u[:], u[:], mr216[:], OP.subtract)
                eng.tensor_scalar(
                    out=outT[:, do, :], in0=u[:],
                    scalar1=g2_c[:, do:do + 1],
                    scalar2=be2_c[:, do:do + 1],
                    op0=OP.mult, op1=OP.add)
                nc.sync.dma_start(out[do], outT[:, do, :])

    nc.finalize()
    return nc


def _get_nc():
    if "nc" not in _CACHE:
        _CACHE["nc"] = _build()
    return _CACHE["nc"]


def _tri():
    t = np.zeros((P, QT_T, QCH), np.float16)
    for j in range(QT_T):
        for p in range(P):
            t[p, j, 128 * j + p:] = 1.0
    return t


def _prep_shared(W1, b1, W2, b2, gamma1, beta1, gamma2, beta2,
                 bq, bk, bo, Wo_unused=None):
    f16 = np.float16
    f32 = np.float32
    W1 = np.asarray(W1, f32)
    W2 = np.asarray(W2, f32)
    g1 = np.asarray(gamma1, f32)
    be1 = np.asarray(beta1, f32)
    W1p = g1[:, None] * W1
    c1 = W1.T @ be1 + np.asarray(b1, f32)
    s1n = -W1p.sum(axis=0)
    shared = {
        "w1": np.ascontiguousarray(
            W1p.reshape(NDT, P, NFT, P).transpose(2, 1, 0, 3).astype(f16)),
        "w2": np.ascontiguousarray(
            W2.reshape(NFT, P, 2, QCH).transpose(2, 0, 1, 3).astype(f16)),
        "ones": np.ones((P, P), dtype=f16),
        "tri": _tri(),
    }
    g2 = np.asarray(gamma2, f32)
    cols8 = np.zeros((P, 10, NDT), f32)
    for idx, vv in enumerate([bq, bk, bo, b2, g1, be1, g2, beta2,
                              -g1, -g2]):
        cols8[:, idx, :] = np.asarray(vv, f32).reshape(NDT, P).T
    shared["cols8"] = cols8
    cols32 = np.zeros((P, 2, NFT), f32)
    cols32[:, 0, :] = c1.reshape(NFT, P).T
    cols32[:, 1, :] = s1n.reshape(NFT, P).T
    shared["cols32"] = cols32
    sel2 = np.zeros((HD, P), f32)
    sel2[0, 0:HD] = 1.0
    sel2[32, HD:P] = 1.0
    shared["sel2"] = sel2
    return shared


def kernel(x, mask, Wq, bq, Wk, bk, Wv, bv, Wo, bo, W1, b1, W2, b2,
           gamma1, beta1, gamma2, beta2, _trace=False):
    from concourse.bass_utils import run_bass_kernel_spmd

    nc = _get_nc()
    f16 = np.float16
    f32 = np.float32
    x = np.asarray(x, f32)
    Wq = np.asarray(Wq, f32)
    Wk = np.asarray(Wk, f32)
    Wv = np.asarray(Wv, f32)
    Wo = np.asarray(Wo, f32)
    bq = np.asarray(bq, f32)
    bk = np.asarray(bk, f32)
    bv = np.asarray(bv, f32)

    shared = _prep_shared(W1, b1, W2, b2, gamma1, beta1, gamma2, beta2,
                          bq, bk, bo)
    # x^T tiles: [tt=b*4+qc, p, ko, t]
    xt = np.ascontiguousarray(
        x.reshape(2, 4, QCH, NDT, P).transpose(0, 1, 4, 3, 2)
        .reshape(NTT, P, NDT, QCH).astype(f16))

    def wslice_cols(W, c):   # [p(=d%128), ko, hd] for head cols of core c
        return np.ascontiguousarray(
            W[:, P * c:P * (c + 1)].reshape(NDT, P, P)
            .transpose(1, 0, 2).astype(f16))

    in_maps = []
    for c in range(NCORES):
        b, r = divmod(c, 4) if c < 4 else (1, c - 4)
        b = c // 4
        r = c % 4
        wo_sl = np.ascontiguousarray(
            Wo[P * c:P * (c + 1), :].reshape(P, NDT, P).astype(f16))
        bqk = np.zeros((P, 2), f32)
        bqk[:, 0] = bq[P * c:P * (c + 1)]
        bqk[:, 1] = bk[P * c:P * (c + 1)]
        bvb = np.zeros((P, P), f16)
        bvb[:, :] = bv[P * c:P * (c + 1)].astype(f16)[None, :]
        # owned tokens: batch b, token 512*qc + 128*r + tl at col 128*qc+tl
        # bo is folded in here so the residual is one tensor_tensor add
        xres = np.empty((P, NDT, QCH), np.float32)
        bo_col = np.asarray(bo, f32).reshape(NDT, P).T[:, :, None]  # [p,do,1]
        for qc in range(4):
            blk = x[b, QCH * qc + P * r: QCH * qc + P * r + P, :]  # [128,1024]
            xres[:, :, P * qc:P * (qc + 1)] = \
                blk.T.reshape(NDT, P, P).transpose(1, 0, 2)
        xres = (xres + bo_col).astype(f16)
        in_maps.append({
            "xt": xt,
            "wq": wslice_cols(Wq, c),
            "wk": wslice_cols(Wk, c),
            "wv": wslice_cols(Wv, c),
            "wo": wo_sl,
            "bqk": bqk,
            "bvb": bvb,
            "xres": np.ascontiguousarray(xres),
            **shared,
        })
    res = run_bass_kernel_spmd(nc, in_maps, core_ids=list(range(NCORES)),
                               trace=_trace)
    outp = np.empty((B, S, D), np.float32)
    for c in range(NCORES):
        b = c // 4
        r = c % 4
        o = np.asarray(res.results[c]["out"], np.float32)  # [8, 128, 512]
        toks = o.transpose(2, 0, 1).reshape(QCH, D)        # rows = qc*128+tl
        for qc in range(4):
            outp[b, QCH * qc + P * r: QCH * qc + P * r + P] = \
                toks[P * qc:P * (qc + 1)]
    if _trace:
        _CACHE["last_result"] = res
    return outp
